# revision 12
# baseline (speedup 1.0000x reference)
"""Conditional 1x1 conv (per-sample class-routed weights) on 8 Trainium2 cores.

Strategy (hardcoded for x:[32,64,64,512] f32, cls:[32,1] int64,
kernel:[120,1,1,512,512] f32, bias:[120,512] f32):

- Host: gather per-sample weight [B,C,F] = kernel[cls], transpose x to
  [B, C, HW] (channels-on-partitions layout so the device needs no
  transposes at all), cast x/w to bf16, shard batch 4-samples-per-core
  across 8 cores.
- Device (per core, SPMD): per 128-pixel tile j, out[p,f] = sum_k
  xT[c,p].T @ w[c,f] accumulated over 4 c-chunks in a 2-bank PSUM pair
  tile; one DVE/Act copy drains 2 matmul groups into a per-block SBUF
  staging tile; ONE SWDGE store per 2048-pixel block writes it out.
- Host: concat core outputs, upcast to f32, reshape back to [B,H,W,F].

Perf notes (measured on TRN2 silicon via in-NEFF rep differentials):
  * bf16 matmul streams ~2 rows/cycle: the 512-matmul stream alone runs
    ~56us/core, and bf16 halves DMA traffic vs f32 (35.6MB/core, ~49us
    standalone) -- so bf16 beats f32/f32r on both axes.
  * HWDGE-issued output stores collide with PE compute (stores+compute
    = 104us even with all dependencies removed); the same stores issued
    through the Pool-engine SWDGE queue compose cleanly (56us). Hence
    ALL bulk DMA (x loads + out stores) rides SWDGE; only the tiny
    per-sample w/bias loads use the SP HWDGE queue.

Accuracy: bf16 in/out gives ~3e-3 rel Frobenius error vs the f32
reference -- an order of magnitude inside the 2e-2 gate.
"""

import numpy as np
import ml_dtypes

import concourse.bacc as bacc
import concourse.mybir as mybir
import concourse.tile as tile
from concourse import bass
from concourse.bass_utils import run_bass_kernel_spmd

B, H, W, C, F = 32, 64, 64, 512, 512
NCORES = 8
SPC = B // NCORES          # samples per core
NPIX = H * W               # 4096 pixels per sample
P = 128                    # partitions
KO = C // P                # 4 contraction chunks
PB = 4096                  # pixel block per x-tile DMA (8 KiB contiguous rows)
NPB = NPIX // PB           # 1 pixel block per sample
PT = PB // P               # 16 pixel tiles per block

BF16 = mybir.dt.bfloat16
NP_BF16 = ml_dtypes.bfloat16

_CACHE: dict = {}
_last_results = None       # test harness introspection


def _build(add_bias: bool, reps: int = 1):
    nc = bacc.Bacc("TRN2", target_bir_lowering=False, debug=False)
    xt_d = nc.declare_dram_parameter("xt", [SPC, C, NPIX], BF16, isOutput=False)
    wt_d = nc.declare_dram_parameter("wt", [SPC, C, F], BF16, isOutput=False)
    if add_bias:
        bt_d = nc.declare_dram_parameter("bt", [SPC, P, F], mybir.dt.float32, isOutput=False)
    # pixel-tile-transposed layout: [s, p, jtile, f] so each partition's
    # store descriptor run is 4KB contiguous (vs 1KB in [s, pix, f]);
    # host un-permutes for free
    out_d = nc.declare_dram_parameter(
        "out", [SPC, P, NPIX // P, F], BF16, isOutput=True
    )

    cp = 0  # copy round-robin counter

    with tile.TileContext(nc) as tc:
        with (
            tc.tile_pool(name="xpool", bufs=3) as xpool,
            tc.tile_pool(name="wpool", bufs=2) as wpool,
            tc.tile_pool(name="opool", bufs=4) as opool,
            tc.tile_pool(name="pspool", bufs=4, space="PSUM") as pspool,
        ):
          for _rep in range(reps):
            for s in range(SPC):
                w_sb = wpool.tile([P, KO, F], BF16, tag="w")
                nc.sync.dma_start(
                    w_sb[:], wt_d[s].rearrange("(ko ki) f -> ki ko f", ki=P)
                )
                if add_bias:
                    b_sb = wpool.tile([P, F], mybir.dt.float32, tag="b")
                    nc.sync.dma_start(b_sb[:], bt_d[s])
                for pb in range(NPB):
                    x_sb = xpool.tile([P, KO, PB], BF16, tag="x")
                    xv = xt_d[s].rearrange("(ko ki) p -> ki ko p", ki=P)
                    # one whole-sample load (512 descs x 8KB) on the SWDGE ring
                    nc.gpsimd.dma_start(x_sb[:], xv)
                    for sb in range(PT // 4):       # 4-tile store batches
                        o_sb = opool.tile([P, 4, F], BF16, tag="o")
                        for pr in range(2):         # pixel-tile pairs
                            ps2 = pspool.tile([P, 2, F], mybir.dt.float32, tag="ps")
                            for jj in range(2):
                                j = sb * 4 + pr * 2 + jj
                                for k in range(KO):
                                    nc.tensor.matmul(
                                        ps2[:, jj, :],
                                        x_sb[:, k, bass.ts(j, P)],
                                        w_sb[:, k, :],
                                        start=(k == 0),
                                        stop=(k == KO - 1),
                                    )
                            dst = o_sb[:, 2 * pr : 2 * pr + 2, :]
                            if add_bias:
                                for jj in range(2):
                                    if cp % 2 == 0:
                                        nc.vector.tensor_tensor(
                                            dst[:, jj, :], ps2[:, jj, :], b_sb[:],
                                            mybir.AluOpType.add,
                                        )
                                    else:
                                        nc.scalar.activation(
                                            out=dst[:, jj, :], in_=ps2[:, jj, :],
                                            func=mybir.ActivationFunctionType.Copy,
                                            bias=b_sb[:],
                                        )
                                    cp += 1
                            else:
                                # Pool/GPSIMD can't read PSUM; alternate DVE/Act
                                if cp % 2 == 0:
                                    nc.vector.tensor_copy(out=dst, in_=ps2[:])
                                else:
                                    nc.scalar.copy(out=dst, in_=ps2[:])
                                cp += 1
                        # stores stay on the SWDGE queue
                        jg = pb * PT + sb * 4
                        nc.gpsimd.dma_start(out_d[s, :, jg : jg + 4, :], o_sb[:])
    nc.compile()
    return nc


def kernel(x, cls, kernel, bias):
    global _last_results
    x = np.asarray(x, dtype=np.float32)
    cls_idx = np.asarray(cls).reshape(-1).astype(np.int64)
    ktab = np.asarray(kernel, dtype=np.float32).reshape(-1, C, F)
    bias = np.asarray(bias, dtype=np.float32)

    # host-side routing + layout prep
    w_all = ktab[cls_idx].astype(NP_BF16)                   # [B, C, F] bf16
    b_all = bias[cls_idx]                                   # [B, F]
    add_bias = bool(np.any(b_all))
    xt_all = np.ascontiguousarray(
        x.reshape(B, NPIX, C).transpose(0, 2, 1)            # [B, C, NPIX]
    ).astype(NP_BF16)

    key = ("cc11", add_bias)
    if key not in _CACHE:
        _CACHE[key] = _build(add_bias)
    nc = _CACHE[key]

    in_maps = []
    for c in range(NCORES):
        sl = slice(c * SPC, (c + 1) * SPC)
        m = {
            "xt": np.ascontiguousarray(xt_all[sl]),
            "wt": np.ascontiguousarray(w_all[sl]),
        }
        if add_bias:
            m["bt"] = np.ascontiguousarray(
                np.broadcast_to(b_all[sl, None, :], (SPC, P, F))
            ).astype(np.float32)
        in_maps.append(m)

    res = run_bass_kernel_spmd(nc, in_maps, list(range(NCORES)))
    _last_results = res

    out = np.concatenate([res.results[c]["out"] for c in range(NCORES)], axis=0)
    # un-permute pixel-tile-transposed device layout [b, p, jtile, f]
    out = out.astype(np.float32).transpose(0, 2, 1, 3)
    return np.ascontiguousarray(out).reshape(B, H, W, F)


# revision 13
# speedup vs baseline: 1.0525x; 1.0525x over previous
"""Conditional 1x1 conv (per-sample class-routed weights) on 8 Trainium2 cores.

Strategy (hardcoded for x:[32,64,64,512] f32, cls:[32,1] int64,
kernel:[120,1,1,512,512] f32, bias:[120,512] f32):

- Host: gather per-sample weight [B,C,F] = kernel[cls], transpose x to
  [B, C, HW] (channels-on-partitions layout so the device needs no
  transposes at all), cast x/w to bf16, shard batch 4-samples-per-core
  across 8 cores.
- Device (per core, SPMD): per 128-pixel tile j, out[p,f] = sum_k
  xT[c,p].T @ w[c,f] accumulated over 4 c-chunks in a 2-bank PSUM pair
  tile; one DVE/Act copy drains 2 matmul groups into a per-block SBUF
  staging tile; ONE SWDGE store per 2048-pixel block writes it out.
- Host: concat core outputs, upcast to f32, reshape back to [B,H,W,F].

Perf notes (measured on TRN2 silicon via in-NEFF rep differentials):
  * bf16 matmul streams ~2 rows/cycle: the 512-matmul stream alone runs
    ~56us/core, and bf16 halves DMA traffic vs f32 (35.6MB/core, ~49us
    standalone) -- so bf16 beats f32/f32r on both axes.
  * HWDGE-issued output stores collide with PE compute (stores+compute
    = 104us even with all dependencies removed); the same stores issued
    through the Pool-engine SWDGE queue compose cleanly (56us). Hence
    ALL bulk DMA (x loads + out stores) rides SWDGE; only the tiny
    per-sample w/bias loads use the SP HWDGE queue.

Accuracy: bf16 in/out gives ~3e-3 rel Frobenius error vs the f32
reference -- an order of magnitude inside the 2e-2 gate.
"""

import numpy as np
import ml_dtypes

import concourse.bacc as bacc
import concourse.mybir as mybir
import concourse.tile as tile
from concourse import bass
from concourse.bass_utils import run_bass_kernel_spmd

B, H, W, C, F = 32, 64, 64, 512, 512
NCORES = 8
SPC = B // NCORES          # samples per core
NPIX = H * W               # 4096 pixels per sample
P = 128                    # partitions
KO = C // P                # 4 contraction chunks
PB = 2048                  # pixel block per x-tile DMA (4 KiB contiguous rows)
NPB = NPIX // PB           # 2 pixel blocks per sample
PT = PB // P               # 16 pixel tiles per block

BF16 = mybir.dt.bfloat16
NP_BF16 = ml_dtypes.bfloat16

_CACHE: dict = {}
_last_results = None       # test harness introspection


def _build(add_bias: bool, reps: int = 1):
    nc = bacc.Bacc("TRN2", target_bir_lowering=False, debug=False)
    xt_d = nc.declare_dram_parameter("xt", [SPC, C, NPIX], BF16, isOutput=False)
    wt_d = nc.declare_dram_parameter("wt", [SPC, C, F], BF16, isOutput=False)
    if add_bias:
        bt_d = nc.declare_dram_parameter("bt", [SPC, P, F], mybir.dt.float32, isOutput=False)
    # pixel-tile-transposed layout: [s, p, jtile, f] so each partition's
    # store descriptor run is 4KB contiguous (vs 1KB in [s, pix, f]);
    # host un-permutes for free
    out_d = nc.declare_dram_parameter(
        "out", [SPC, P, NPIX // P, F], BF16, isOutput=True
    )

    cp = 0  # copy round-robin counter

    with tile.TileContext(nc) as tc:
        with (
            tc.tile_pool(name="xpool", bufs=6) as xpool,
            tc.tile_pool(name="wpool", bufs=2) as wpool,
            tc.tile_pool(name="opool", bufs=8) as opool,
            tc.tile_pool(name="pspool", bufs=4, space="PSUM") as pspool,
        ):
          for _rep in range(reps):
            for s in range(SPC):
                w_sb = wpool.tile([P, KO, F], BF16, tag="w")
                nc.sync.dma_start(
                    w_sb[:], wt_d[s].rearrange("(ko ki) f -> ki ko f", ki=P)
                )
                if add_bias:
                    b_sb = wpool.tile([P, F], mybir.dt.float32, tag="b")
                    nc.sync.dma_start(b_sb[:], bt_d[s])
                for pb in range(NPB):
                    x_sb = xpool.tile([P, KO, PB], BF16, tag="x")
                    xv = xt_d[s].rearrange("(ko ki) p -> ki ko p", ki=P)[
                        :, :, pb * PB : (pb + 1) * PB
                    ]
                    # x loads on their own HWDGE queue (sync), away from stores
                    nc.sync.dma_start(x_sb[:], xv)
                    for sb in range(PT // 4):       # 4-tile store batches
                        o_sb = opool.tile([P, 4, F], BF16, tag="o")
                        for pr in range(2):         # pixel-tile pairs
                            ps2 = pspool.tile([P, 2, F], mybir.dt.float32, tag="ps")
                            for jj in range(2):
                                j = sb * 4 + pr * 2 + jj
                                for k in range(KO):
                                    nc.tensor.matmul(
                                        ps2[:, jj, :],
                                        x_sb[:, k, bass.ts(j, P)],
                                        w_sb[:, k, :],
                                        start=(k == 0),
                                        stop=(k == KO - 1),
                                    )
                            dst = o_sb[:, 2 * pr : 2 * pr + 2, :]
                            if add_bias:
                                for jj in range(2):
                                    if cp % 2 == 0:
                                        nc.vector.tensor_tensor(
                                            dst[:, jj, :], ps2[:, jj, :], b_sb[:],
                                            mybir.AluOpType.add,
                                        )
                                    else:
                                        nc.scalar.activation(
                                            out=dst[:, jj, :], in_=ps2[:, jj, :],
                                            func=mybir.ActivationFunctionType.Copy,
                                            bias=b_sb[:],
                                        )
                                    cp += 1
                            else:
                                # Pool/GPSIMD can't read PSUM; alternate DVE/Act
                                if cp % 2 == 0:
                                    nc.vector.tensor_copy(out=dst, in_=ps2[:])
                                else:
                                    nc.scalar.copy(out=dst, in_=ps2[:])
                                cp += 1
                        # stores split across SWDGE and Act HWDGE queues
                        jg = pb * PT + sb * 4
                        st_eng = nc.gpsimd if sb % 2 == 0 else nc.scalar
                        st_eng.dma_start(out_d[s, :, jg : jg + 4, :], o_sb[:])
    nc.compile()
    return nc


def kernel(x, cls, kernel, bias):
    global _last_results
    x = np.asarray(x, dtype=np.float32)
    cls_idx = np.asarray(cls).reshape(-1).astype(np.int64)
    ktab = np.asarray(kernel, dtype=np.float32).reshape(-1, C, F)
    bias = np.asarray(bias, dtype=np.float32)

    # host-side routing + layout prep
    w_all = ktab[cls_idx].astype(NP_BF16)                   # [B, C, F] bf16
    b_all = bias[cls_idx]                                   # [B, F]
    add_bias = bool(np.any(b_all))
    xt_all = np.ascontiguousarray(
        x.reshape(B, NPIX, C).transpose(0, 2, 1)            # [B, C, NPIX]
    ).astype(NP_BF16)

    key = ("cc11", add_bias)
    if key not in _CACHE:
        _CACHE[key] = _build(add_bias)
    nc = _CACHE[key]

    in_maps = []
    for c in range(NCORES):
        sl = slice(c * SPC, (c + 1) * SPC)
        m = {
            "xt": np.ascontiguousarray(xt_all[sl]),
            "wt": np.ascontiguousarray(w_all[sl]),
        }
        if add_bias:
            m["bt"] = np.ascontiguousarray(
                np.broadcast_to(b_all[sl, None, :], (SPC, P, F))
            ).astype(np.float32)
        in_maps.append(m)

    res = run_bass_kernel_spmd(nc, in_maps, list(range(NCORES)))
    _last_results = res

    out = np.concatenate([res.results[c]["out"] for c in range(NCORES)], axis=0)
    # un-permute pixel-tile-transposed device layout [b, p, jtile, f]
    out = out.astype(np.float32).transpose(0, 2, 1, 3)
    return np.ascontiguousarray(out).reshape(B, H, W, F)


# revision 14
# speedup vs baseline: 1.0802x; 1.0262x over previous
"""Conditional 1x1 conv (per-sample class-routed weights) on 8 Trainium2 cores.

Strategy (hardcoded for x:[32,64,64,512] f32, cls:[32,1] int64,
kernel:[120,1,1,512,512] f32, bias:[120,512] f32):

- Host: gather per-sample weight [B,C,F] = kernel[cls], transpose x to
  [B, C, HW] (channels-on-partitions layout so the device needs no
  transposes at all), cast x/w to bf16, shard batch 4-samples-per-core
  across 8 cores.
- Device (per core, SPMD): per 128-pixel tile j, out[p,f] = sum_k
  xT[c,p].T @ w[c,f] accumulated over 4 c-chunks in a 2-bank PSUM pair
  tile; one DVE/Act copy drains 2 matmul groups into a per-block SBUF
  staging tile; ONE SWDGE store per 2048-pixel block writes it out.
- Host: concat core outputs, upcast to f32, reshape back to [B,H,W,F].

Perf notes (measured on TRN2 silicon via in-NEFF rep differentials):
  * bf16 matmul streams ~2 rows/cycle: the 512-matmul stream alone runs
    ~56us/core, and bf16 halves DMA traffic vs f32 (35.6MB/core, ~49us
    standalone) -- so bf16 beats f32/f32r on both axes.
  * HWDGE-issued output stores collide with PE compute (stores+compute
    = 104us even with all dependencies removed); the same stores issued
    through the Pool-engine SWDGE queue compose cleanly (56us). Hence
    ALL bulk DMA (x loads + out stores) rides SWDGE; only the tiny
    per-sample w/bias loads use the SP HWDGE queue.

Accuracy: bf16 in/out gives ~3e-3 rel Frobenius error vs the f32
reference -- an order of magnitude inside the 2e-2 gate.
"""

import numpy as np
import ml_dtypes

import concourse.bacc as bacc
import concourse.mybir as mybir
import concourse.tile as tile
from concourse import bass
from concourse.bass_utils import run_bass_kernel_spmd

B, H, W, C, F = 32, 64, 64, 512, 512
NCORES = 8
SPC = B // NCORES          # samples per core
NPIX = H * W               # 4096 pixels per sample
P = 128                    # partitions
KO = C // P                # 4 contraction chunks
PB = 2048                  # pixel block per x-tile DMA (4 KiB contiguous rows)
NPB = NPIX // PB           # 2 pixel blocks per sample
PT = PB // P               # 16 pixel tiles per block

BF16 = mybir.dt.bfloat16
NP_BF16 = ml_dtypes.bfloat16

_CACHE: dict = {}
_last_results = None       # test harness introspection


def _build(add_bias: bool, reps: int = 1):
    nc = bacc.Bacc("TRN2", target_bir_lowering=False, debug=False)
    xt_d = nc.declare_dram_parameter("xt", [SPC, C, NPIX], BF16, isOutput=False)
    wt_d = nc.declare_dram_parameter("wt", [SPC, C, F], BF16, isOutput=False)
    if add_bias:
        bt_d = nc.declare_dram_parameter("bt", [SPC, P, F], mybir.dt.float32, isOutput=False)
    # pixel-tile-transposed layout: [s, p, jtile, f] so each partition's
    # store descriptor run is 4KB contiguous (vs 1KB in [s, pix, f]);
    # host un-permutes for free
    out_d = nc.declare_dram_parameter(
        "out", [SPC, P, NPIX // P, F], BF16, isOutput=True
    )

    cp = 0  # copy round-robin counter

    with tile.TileContext(nc) as tc:
        with (
            tc.tile_pool(name="xpool", bufs=6) as xpool,
            tc.tile_pool(name="wpool", bufs=2) as wpool,
            tc.tile_pool(name="opool", bufs=8) as opool,
            tc.tile_pool(name="zpool", bufs=1) as zpool,
            tc.tile_pool(name="pspool", bufs=4, space="PSUM") as pspool,
        ):
          z_sb = zpool.tile([P, 2, F], mybir.dt.float32, tag="z")
          nc.vector.memset(z_sb[:], 0.0)
          for _rep in range(reps):
            for s in range(SPC):
                w_sb = wpool.tile([P, KO, F], BF16, tag="w")
                nc.sync.dma_start(
                    w_sb[:], wt_d[s].rearrange("(ko ki) f -> ki ko f", ki=P)
                )
                if add_bias:
                    b_sb = wpool.tile([P, F], mybir.dt.float32, tag="b")
                    nc.sync.dma_start(b_sb[:], bt_d[s])
                for pb in range(NPB):
                    x_sb = xpool.tile([P, KO, PB], BF16, tag="x")
                    xv = xt_d[s].rearrange("(ko ki) p -> ki ko p", ki=P)[
                        :, :, pb * PB : (pb + 1) * PB
                    ]
                    # HWDGE never contends with engines; split across queues
                    nc.sync.dma_start(x_sb[:, : KO // 2, :], xv[:, : KO // 2, :])
                    nc.scalar.dma_start(x_sb[:, KO // 2 :, :], xv[:, KO // 2 :, :])
                    for sb in range(PT // 4):       # 4-tile store batches
                        o_sb = opool.tile([P, 4, F], BF16, tag="o")
                        for pr in range(2):         # pixel-tile pairs
                            ps2 = pspool.tile([P, 2, F], mybir.dt.float32, tag="ps")
                            for jj in range(2):
                                j = sb * 4 + pr * 2 + jj
                                for k in range(KO):
                                    nc.tensor.matmul(
                                        ps2[:, jj, :],
                                        x_sb[:, k, bass.ts(j, P)],
                                        w_sb[:, k, :],
                                        start=(k == 0),
                                        stop=(k == KO - 1),
                                    )
                            dst = o_sb[:, 2 * pr : 2 * pr + 2, :]
                            if add_bias:
                                for jj in range(2):
                                    if cp % 2 == 0:
                                        nc.vector.tensor_tensor(
                                            dst[:, jj, :], ps2[:, jj, :], b_sb[:],
                                            mybir.AluOpType.add,
                                        )
                                    else:
                                        nc.scalar.activation(
                                            out=dst[:, jj, :], in_=ps2[:, jj, :],
                                            func=mybir.ActivationFunctionType.Copy,
                                            bias=b_sb[:],
                                        )
                                    cp += 1
                            else:
                                # DVE tensor_tensor (1-port mode: never grabs
                                # the shared GpSimd port); Act has own ports
                                if cp % 2 == 0:
                                    nc.vector.tensor_tensor(
                                        dst, ps2[:], z_sb[:], mybir.AluOpType.add
                                    )
                                else:
                                    nc.scalar.copy(out=dst, in_=ps2[:])
                                cp += 1
                        # stores on HWDGE queues only (no SWDGE anywhere)
                        jg = pb * PT + sb * 4
                        st_eng = nc.sync if sb % 2 == 0 else nc.scalar
                        st_eng.dma_start(out_d[s, :, jg : jg + 4, :], o_sb[:])
    nc.compile()
    return nc


def kernel(x, cls, kernel, bias):
    global _last_results
    x = np.asarray(x, dtype=np.float32)
    cls_idx = np.asarray(cls).reshape(-1).astype(np.int64)
    ktab = np.asarray(kernel, dtype=np.float32).reshape(-1, C, F)
    bias = np.asarray(bias, dtype=np.float32)

    # host-side routing + layout prep
    w_all = ktab[cls_idx].astype(NP_BF16)                   # [B, C, F] bf16
    b_all = bias[cls_idx]                                   # [B, F]
    add_bias = bool(np.any(b_all))
    xt_all = np.ascontiguousarray(
        x.reshape(B, NPIX, C).transpose(0, 2, 1)            # [B, C, NPIX]
    ).astype(NP_BF16)

    key = ("cc11", add_bias)
    if key not in _CACHE:
        _CACHE[key] = _build(add_bias)
    nc = _CACHE[key]

    in_maps = []
    for c in range(NCORES):
        sl = slice(c * SPC, (c + 1) * SPC)
        m = {
            "xt": np.ascontiguousarray(xt_all[sl]),
            "wt": np.ascontiguousarray(w_all[sl]),
        }
        if add_bias:
            m["bt"] = np.ascontiguousarray(
                np.broadcast_to(b_all[sl, None, :], (SPC, P, F))
            ).astype(np.float32)
        in_maps.append(m)

    res = run_bass_kernel_spmd(nc, in_maps, list(range(NCORES)))
    _last_results = res

    out = np.concatenate([res.results[c]["out"] for c in range(NCORES)], axis=0)
    # un-permute pixel-tile-transposed device layout [b, p, jtile, f]
    out = out.astype(np.float32).transpose(0, 2, 1, 3)
    return np.ascontiguousarray(out).reshape(B, H, W, F)
